# revision 26
# baseline (speedup 1.0000x reference)
"""Trainium2 Bass kernel for the batched contrastive (NT-Xent-style) loss.

Problem (hardcoded shapes): z1, z2: [4, 256, 64, 64] f32.
  h = transpose(reshape(z, [4, 256, 4096]))        # [b, n=4096, c=256]
  a, b = l2-normalize rows of h1, h2
  semi(x, y): refl = exp(x@x^T/tau); between = exp(x@y^T/tau)
              loss_i = -log(between_ii / (refl_sum_i + between_sum_i - refl_ii))
  out = mean((semi(a,b) + semi(b,a))/2)

Per batch element the device needs only:
  sA_i = rowsum exp(a@a^T/tau), sB_i = rowsum exp(b@b^T/tau),
  sC_i = rowsum exp(a@b^T/tau), tC_j = colsum exp(a@b^T/tau),
  dots_i = a_i.b_i/tau
Then l1 = log(sA+sC-e^{1/tau}) - dots, l2 = log(sB+tC-e^{1/tau}) - dots.

Sharding: 8 cores = 4 batch elements x 2 row-halves. Each core receives a
concat input [z[b] | z[b][:, half]] of shape [256, 6144] per side so the
compiled SPMD program is identical across cores: lhsT tiles come from the
trailing 2048 columns (this core's output rows), rhs from the leading 4096.
Both operands are normalized on-device and pre-scaled by 1/sqrt(tau) so the
Gram matmul directly produces the exp() argument.
"""

import ml_dtypes
import numpy as np

import concourse.bacc as bacc
import concourse.bass as bass  # noqa: F401  (MemorySpace etc.)
import concourse.bass_isa as bass_isa
import concourse.mybir as mybir
import concourse.tile as tile
from concourse.bass_utils import run_bass_kernel_spmd

TAU = 0.4
P = 128          # partitions
C = 256          # channels (contraction dim) = 2 k-tiles
KT = 2
NF = 4096        # n (full columns)
NH = 2048        # rows per core
NCAT = NF + NH   # 6144
CH = 512         # matmul free-dim chunk
STRIPE = 1024    # psum stripe width (2 banks; 4 bufs pipeline)
F32 = mybir.dt.float32
BF16 = mybir.dt.bfloat16

# The A and B Gram matrices are symmetric, so each core computes only a
# uniform set of 1024x1024 blocks; the per-core rhs COLUMN PERMUTATION
# (chosen by the host) makes the same compiled block list cover every
# unordered block pair of A (and of B) exactly once across the core pair:
#   rhs slots s0..s3 = global 1024-col blocks PI[h] (h0: [0,1,2,3],
#   h1: [2,3,1,0]); lhsT rows L0, L1 = own global blocks 2h, 2h+1.
#   blocks = (L0,s0)diag, (L1,s1)diag, (L0,s1), (L0,s2), (L1,s3)
# Off-diagonal blocks also accumulate column sums (the transposed block's
# row sums); the host adds them into the right global rows.
#
# out layout (fp32, 18432):
#  [0:1024)       sA rowsum partials for L0 rows: dram[p*8 + I]
#  [1024:2048)    sA partials L1, same layout
#  [2048:4096)    sB partials, L0 | L1
#  [4096:6144)    sC full rowsums, stored as [128, 16]: dram[p*16 + I]
#  [6144:10240)   csC partial colsums (permuted slot order)
#  [10240:12288)  dots (a_i.b_i)/tau, own rows natural order
#  [12288:15360)  csA colsum harvests for slots s1, s2, s3
#  [15360:18432)  csB same
OUT_SIZE = 3 * NH + NF + NH + 2 * 3 * 1024  # 18432
BLK = 1024
# (lhsT block, rhs slot, needs colsum accumulation)
AB_BLOCKS = ((0, 0, False), (1, 1, False), (0, 1, True), (0, 2, True), (1, 3, True))
# rowsum accumulator column ordinal within each lhsT row-block
AB_ORD = {(0, 0): 0, (0, 1): 1, (0, 2): 2, (1, 1): 0, (1, 3): 1}
AB_NBLK = {0: 3, 1: 2}  # blocks per lhsT row-block
# colsum accumulator region per off-diagonal block
AB_REGION = {(0, 1): 0, (0, 2): 1, (1, 3): 2}

_PROGRAM = None


def _build_program():
    nc = bacc.Bacc(
        "TRN2",
        target_bir_lowering=False,
        debug=False,
        enable_asserts=False,
        num_devices=8,
    )
    zc1 = nc.dram_tensor("zc1", [C, NCAT], BF16, kind="ExternalInput")
    zc2 = nc.dram_tensor("zc2", [C, NCAT], BF16, kind="ExternalInput")
    out_t = nc.dram_tensor("out", [OUT_SIZE], F32, kind="ExternalOutput")

    Act = mybir.ActivationFunctionType

    # The input is processed in three 2048-column pieces; the lhsT region
    # (piece 2, chunks 8..11) is loaded and normalized first so the A product
    # can start early. Emission order is:
    #   norm(z1) -> product A -> norm(z2) -> dots -> product C (+colsums)
    #   -> product B
    # which keeps ScalarE busy with A's exps while z2 streams in/normalizes,
    # and hides C's colsum finalization under B. ScalarE alternates between
    # the sqrt and exp table sets only ~4 times.
    ZPIECES = (4, 0, 5, 1, 2, 3)  # 1024-col load order: lhsT-a, rhs0, lhsT-b, ...

    with tile.TileContext(nc) as tc:
        with (
            tc.tile_pool(name="zstage", bufs=12) as zpool,
            tc.tile_pool(name="sqpool", bufs=2) as sqpool,
            tc.tile_pool(name="abpool", bufs=1) as abpool,
            tc.tile_pool(name="rwpool", bufs=3) as rwpool,
            tc.tile_pool(name="ecpool", bufs=6) as ecpool,
            tc.tile_pool(name="accpool", bufs=1) as accpool,
            tc.tile_pool(name="pspool", bufs=4, space="PSUM") as pspool,
        ):
            # constants
            ones_bf = accpool.tile([P, P], BF16, name="ones_bf")
            nc.vector.memset(ones_bf, 1.0)

            # persistent normalized operands (scaled by 1/sqrt(tau)), bf16
            a_sb = [abpool.tile([P, NCAT], BF16, name=f"a{k}") for k in range(KT)]
            b_sb = [abpool.tile([P, NCAT], BF16, name=f"b{k}") for k in range(KT)]
            cacc = accpool.tile([P, NF], BF16, name="cacc")
            rs = {"C": accpool.tile([P, 64], F32, name="rsC")}

            def norm_load(zdram):
                """DMA z [256, 6144] fp32 in 1024-col pieces (lhsT pieces
                early) and square each piece (DVE/GPSIMD alternating)."""
                zts = {}
                sqs = []
                for k in range(KT):
                    sqs.append(
                        sqpool.tile([P, NCAT], BF16, tag="sq", name=f"sq{k}")
                    )
                for p in ZPIECES:
                    sl = slice(p * BLK, (p + 1) * BLK)
                    for k in range(KT):
                        zp = zpool.tile([P, BLK], BF16, tag="z", name=f"z{k}_{p}")
                        nc.sync.dma_start(
                            out=zp, in_=zdram[k * P : (k + 1) * P, sl]
                        )
                        eng = nc.vector if (k + p) % 2 == 0 else nc.gpsimd
                        eng.tensor_mul(sqs[k][:, sl], zp, zp)
                        zts[(k, p)] = zp
                return zts, sqs

            def norm_groups(zts, sqs, dst, groups):
                """Per chunk group: column-sums of z^2 via all-ones matmul
                (broadcast to all partitions), then a single ScalarE
                rsqrt(tau*x) straight from PSUM, then dst = z * rnorm."""
                for chunks in groups:
                    rw = rwpool.tile([P, len(chunks) * CH], F32, tag="rw", name="rw")
                    for slot, ch in enumerate(chunks):
                        sl = slice(ch * CH, (ch + 1) * CH)
                        psn = pspool.tile([P, CH], F32, tag="ps", name="psn")
                        for k in range(KT):
                            nc.tensor.matmul(
                                psn,
                                ones_bf,
                                sqs[k][:, sl],
                                start=(k == 0),
                                stop=(k == KT - 1),
                            )
                        nc.vector.reciprocal(rw[:, slot * CH : (slot + 1) * CH], psn)
                    nc.scalar.activation(out=rw, in_=rw, func=Act.Sqrt, scale=1.0 / TAU)
                    for slot, ch in enumerate(chunks):
                        p, off = ch // 2, (ch % 2) * CH
                        sl = slice(ch * CH, (ch + 1) * CH)
                        rsl = slice(slot * CH, (slot + 1) * CH)
                        for k in range(KT):
                            eng = nc.vector if (k + ch) % 2 == 0 else nc.gpsimd
                            eng.tensor_mul(
                                dst[k][:, sl],
                                zts[(k, p)][:, off : off + CH],
                                rw[:, rsl],
                            )

            def colsum_out(src, n, off):
                """Partition-reduce src [128, n] -> DRAM out_t[off:off+n] via
                ones-column PE matmuls (replaces gpsimd.partition_all_reduce,
                which costs ~8us per call on HW): psum [1,512] chunks, DVE
                copy into one SBUF row, single DMA out."""
                row = rwpool.tile([1, n], F32, tag="csrow", name="csrow")
                for c0 in range(0, n, CH):
                    pc = pspool.tile([1, CH], F32, tag="ps", name="psr")
                    nc.tensor.matmul(
                        pc, ones_bf[:, 0:1], src[:, c0 : c0 + CH],
                        start=True, stop=True,
                    )
                    nc.vector.tensor_copy(row[:, c0 : c0 + CH], pc)
                nc.sync.dma_start(out=out_t[off : off + n], in_=row[0:1, :])

            # A/B rowsum accumulators: rs_ab[(prod, lr)]; col = I*nblk + ord
            rs_ab = {
                (m, lr): accpool.tile(
                    [P, 8 * AB_NBLK[lr]], F32, name=f"rs{m}{lr}"
                )
                for m in ("A", "B")
                for lr in (0, 1)
            }
            # A/B colsum-harvest accumulators over slot regions s1,s2,s3
            # (bf16: halves the DVE/GPSIMD read-modify-write traffic; the
            # values are O(100) sums of exp<=e^2.5, well within tolerance)
            acc_ab = {
                m: accpool.tile([P, 3 * BLK], BF16, name=f"acc{m}")
                for m in ("A", "B")
            }

            def ab_block(pname, t_sb, lr, s, colacc):
                """One 1024x1024 symmetric-product block: lhsT row-block lr,
                rhs slot s. exp + rowsum fused on ScalarE; off-diagonal blocks
                also accumulate column sums via DMA-CCE (SWDGE accumulate):
                the Q7 only pays ~1us of descriptor-gen per tile while the
                actual read-modify-write runs on the idle DMA engines."""
                for I in range(BLK // P):  # 8
                    lo = NF + lr * BLK + I * P
                    ps = pspool.tile([P, BLK], F32, tag="ps", name="ps_ab")
                    for j2 in range(BLK // CH):  # 2
                        osl = slice(j2 * CH, (j2 + 1) * CH)
                        col = s * BLK + j2 * CH
                        for k in range(KT):
                            nc.tensor.matmul(
                                ps[:, osl],
                                t_sb[k][:, lo : lo + P],
                                t_sb[k][:, col : col + CH],
                                start=(k == 0),
                                stop=(k == KT - 1),
                            )
                    ci = I * AB_NBLK[lr] + AB_ORD[(lr, s)]
                    col_acc = rs_ab[(pname, lr)][:, ci : ci + 1]
                    if colacc:
                        e = ecpool.tile([P, BLK], BF16, tag="ec", name="eab")
                        nc.scalar.activation(
                            out=e, in_=ps, func=Act.Exp, accum_out=col_acc
                        )
                        r = AB_REGION[(lr, s)]
                        asl = slice(r * BLK, (r + 1) * BLK)
                        acc_eng = nc.vector if r == 1 else nc.gpsimd
                        if I == 0:
                            acc_eng.tensor_copy(acc_ab[pname][:, asl], e)
                        else:
                            acc_eng.tensor_add(
                                acc_ab[pname][:, asl], acc_ab[pname][:, asl], e
                            )
                    else:
                        nc.scalar.activation(
                            out=ps, in_=ps, func=Act.Exp, accum_out=col_acc
                        )

            def ab_finalize(pname, colsums=True):
                # rowsum partials: reduce each row-block's accumulator
                off0 = {"A": 0, "B": NH}[pname]
                for lr in (0, 1):
                    nb = AB_NBLK[lr]
                    sf = accpool.tile([P, 8], F32, name=f"sf{pname}{lr}")
                    nc.vector.tensor_reduce(
                        sf,
                        rs_ab[(pname, lr)].rearrange("p (i b) -> p i b", b=nb),
                        axis=mybir.AxisListType.X,
                        op=mybir.AluOpType.add,
                    )
                    o = off0 + lr * BLK
                    nc.sync.dma_start(
                        out=out_t[o : o + BLK].rearrange("(p i) -> p i", i=8),
                        in_=sf,
                    )
                if not colsums:
                    return
                # colsum harvests: PE partition-reduce each slot region
                cs0 = {"A": 6 * NH, "B": 6 * NH + 3 * BLK}[pname]
                for r in range(3):
                    colsum_out(
                        acc_ab[pname][:, r * BLK : (r + 1) * BLK], BLK, cs0 + r * BLK
                    )

            def do_c_product():
                """C = a@b^T, full rows x cols, 2048-wide psum stripes,
                h-inner so the two colsum accumulators (DVE for h0, GPSIMD
                for h1) each keep up with ScalarE's exp pace."""
                for I in range(NH // P):  # 16
                    for h in range(NF // STRIPE):  # 2
                        lo = NF + I * P
                        ps = pspool.tile([P, STRIPE], F32, tag="ps", name="ps_mm")
                        for j4 in range(STRIPE // CH):  # 4
                            osl = slice(j4 * CH, (j4 + 1) * CH)
                            col = h * STRIPE + j4 * CH
                            for k in range(KT):
                                nc.tensor.matmul(
                                    ps[:, osl],
                                    a_sb[k][:, lo : lo + P],
                                    b_sb[k][:, col : col + CH],
                                    start=(k == 0),
                                    stop=(k == KT - 1),
                                )
                        col_acc = rs["C"][:, I * 4 + h : I * 4 + h + 1]
                        e = ecpool.tile([P, STRIPE], BF16, tag="ec", name="ec")
                        nc.scalar.activation(
                            out=e, in_=ps, func=Act.Exp, accum_out=col_acc
                        )
                        csl = slice(h * STRIPE, (h + 1) * STRIPE)
                        eng = nc.vector if h < 2 else nc.gpsimd
                        if I == 0:
                            eng.tensor_copy(cacc[:, csl], e)
                        else:
                            eng.tensor_add(cacc[:, csl], cacc[:, csl], e)
                # full rowsums: rs["C"] is [128, 16 I x 2 h] -> sum the pairs
                sf = accpool.tile([P, 16], F32, name="sfinC")
                nc.vector.tensor_reduce(
                    sf,
                    rs["C"].rearrange("p (i h) -> p i h", h=4),
                    axis=mybir.AxisListType.X,
                    op=mybir.AluOpType.add,
                )
                nc.sync.dma_start(
                    out=out_t[2 * NH : 3 * NH].rearrange("(p i) -> p i", i=16),
                    in_=sf,
                )

            # side a fully normalized first (all sqrts precede all exps on
            # ScalarE, minimizing activation-table switches), then all A
            # blocks; side b normalizes while A's exps keep ScalarE busy.
            zta, sqa = norm_load(zc1)
            norm_groups(
                zta, sqa, a_sb, [[8, 9], [0, 1], [10, 11], [2, 3], [4, 5], [6, 7]]
            )
            ab_block("A", a_sb, 0, 0, False)
            ab_block("A", a_sb, 1, 1, False)
            ab_block("A", a_sb, 0, 1, True)
            ztb, sqb = norm_load(zc2)
            ab_block("A", a_sb, 0, 2, True)
            norm_groups(
                ztb, sqb, b_sb, [[8, 9, 10, 11], [0, 1, 2, 3], [4, 5, 6, 7]]
            )
            ab_block("A", a_sb, 1, 3, True)
            # rowsum partials only; the colsum harvests are deferred past the
            # C product so their DVE copies don't contend with C's add chains
            ab_finalize("A", colsums=False)

            do_c_product()
            # drain-phase reductions: A harvests, exp(C) colsum partials, and
            # dots all land here, overlapping with the B product.
            cs0a = 6 * NH
            for r in range(3):
                colsum_out(
                    acc_ab["A"][:, r * BLK : (r + 1) * BLK], BLK, cs0a + r * BLK
                )
            for half in range(2):
                colsum_out(
                    cacc[:, half * NH : (half + 1) * NH], NH, 3 * NH + half * NH
                )

            # B: colacc blocks first so the colsum reduces hide under the
            # diagonal blocks' exps at the tail.
            for lr, s, colacc in ((0, 1, True), (0, 2, True), (1, 3, True)):
                ab_block("B", b_sb, lr, s, colacc)
                r = AB_REGION[(lr, s)]
                cs0 = 6 * NH + 3 * BLK
                colsum_out(
                    acc_ab["B"][:, r * BLK : (r + 1) * BLK], BLK, cs0 + r * BLK
                )
            ab_block("B", b_sb, 0, 0, False)
            ab_block("B", b_sb, 1, 1, False)
            ab_finalize("B", colsums=False)

            # dots (tiny) at the very end: DVE is idle during the drain of
            # B's diagonal exps, so these muls and the PE reduce hide there.
            dm0 = ecpool.tile([P, NH], BF16, tag="ec", name="dm0")
            dm1 = ecpool.tile([P, NH], BF16, tag="ec", name="dm1")
            nc.vector.tensor_mul(dm0, a_sb[0][:, NF:], b_sb[0][:, NF:])
            nc.vector.tensor_mul(dm1, a_sb[1][:, NF:], b_sb[1][:, NF:])
            nc.vector.tensor_add(dm0, dm0, dm1)
            colsum_out(dm0, NH, 3 * NH + NF)

    nc.compile()
    return nc


def _get_program():
    global _PROGRAM
    if _PROGRAM is None:
        _PROGRAM = _build_program()
    return _PROGRAM


# per-core rhs slot permutation: slot s holds global 1024-col block PI[h][s]
PI = ((0, 1, 2, 3), (2, 3, 1, 0))


def _make_zc(z, half):
    """[256, 4096] bf16 -> [256, 6144] = permuted rhs | own lhsT half."""
    perm = PI[half]
    rhs = np.concatenate([z[:, g * BLK : (g + 1) * BLK] for g in perm], axis=1)
    return np.concatenate([rhs, z[:, half * NH : (half + 1) * NH]], axis=1)


def _run_cores(z1, z2, **run_kwargs):
    """Shard, run the SPMD program on 8 cores, return per-core result dicts."""
    nc = _get_program()
    z1 = np.asarray(z1, dtype=np.float32).reshape(4, C, NF).astype(ml_dtypes.bfloat16)
    z2 = np.asarray(z2, dtype=np.float32).reshape(4, C, NF).astype(ml_dtypes.bfloat16)
    in_maps = []
    for core in range(8):
        b, half = core // 2, core % 2
        in_maps.append(
            {"zc1": _make_zc(z1[b], half), "zc2": _make_zc(z2[b], half)}
        )
    res = run_bass_kernel_spmd(nc, in_maps, list(range(8)), **run_kwargs)
    return res


def _combine(results):
    """Host-side final math: tiny [4096]-vector ops + mean."""
    e0 = np.exp(1.0 / TAU)
    losses = []
    for b in range(4):
        parts = [
            np.asarray(results[2 * b + h]["out"], dtype=np.float64) for h in (0, 1)
        ]

        def rsum(region):  # [1024] rowsum partial stored as [128, 8]
            return region.reshape(P, 8).T.reshape(-1)

        def asm(rs_off, cs_off):
            # assemble a symmetric product's full rowsums from the block
            # rowsum partials + transposed-block colsum harvests
            rsl = [
                [rsum(p[rs_off + lr * BLK : rs_off + (lr + 1) * BLK]) for lr in (0, 1)]
                for p in parts
            ]
            cs = [p[cs_off : cs_off + 3 * BLK] for p in parts]
            g = np.empty(NF)
            g[0:BLK] = rsl[0][0] + cs[1][2 * BLK : 3 * BLK]
            g[BLK : 2 * BLK] = rsl[0][1] + cs[0][0:BLK] + cs[1][BLK : 2 * BLK]
            g[2 * BLK : 3 * BLK] = rsl[1][0] + cs[0][BLK : 2 * BLK]
            g[3 * BLK : 4 * BLK] = rsl[1][1] + cs[0][2 * BLK :] + cs[1][0:BLK]
            return g

        sA = asm(0, 6 * NH)
        sB = asm(NH, 6 * NH + 3 * BLK)
        sC = np.concatenate(
            [p[2 * NH : 3 * NH].reshape(P, 16).T.reshape(-1) for p in parts]
        )
        dots = np.concatenate([p[3 * NH + NF : 3 * NH + NF + NH] for p in parts])
        tC = np.zeros(NF)
        for h, p in enumerate(parts):
            for s in range(4):
                g = PI[h][s]
                tC[g * BLK : (g + 1) * BLK] += p[
                    3 * NH + s * BLK : 3 * NH + (s + 1) * BLK
                ]
        l1 = np.log(sA + sC - e0) - dots
        l2 = np.log(sB + tC - e0) - dots
        losses.append(0.5 * (l1 + l2))
    return np.array(np.mean(losses), dtype=np.float32)


def kernel(z1, z2):
    results = _run_cores(z1, z2).results
    return _combine(results)

